# revision 68
# baseline (speedup 1.0000x reference)
"""Trainium2 Bass kernel for CrossMultiHeadedSelfAttention.

Problem: B=2, SQ=SK=2048, D=1024, H=16, HD=64 cross-attention
  q = x @ Wq + bq ; k = enc @ Wk + bk ; v = enc @ Wv + bv   (per head)
  out = softmax(q k^T / sqrt(HD)) v  -> concat heads -> @ Wo + bo

Sharding: 8 cores = 2 batches x 4 head-groups (4 heads per core).
Each core computes a partial output projection over its 4 heads; the host
sums the 4 partials per batch and adds the constant term
(bo + sum_h bv_h @ Wo_h, exact because softmax rows sum to 1).

Device-side math (per core, bf16 matmuls, f32 accumulation):
  - x/enc are pre-transposed AND pre-cast to bf16 on the host, so xT/encT
    d-major tiles load with fully contiguous DMA
  - qT/kT in [head-pair e (128) x seq] layout, bias via per-partition
    tensor_scalar add; v in natural [s, 4*65] layout with a ones column
    per head (gives softmax row-sums for free in the attn@v matmul)
  - scoresT chunk = kT_h.T @ qT_h  -> exp (scale=1/8, no max subtraction:
    scores ~ N(0,1), |s|max ~ 6 so exp is safe in f32/bf16)
  - outU = v'_h.T @ expT  ([65 x 512] in PSUM, row 64 = softmax row-sum)
  - normalize without any transpose: reciprocal of row 64 -> tiny
    partition-broadcast DMA [1,512]->[64,512] -> one tensor_mul writes the
    normalized e-major tile into the pair-stacked stk buffer
  - y = sum_pairs stk_pair.T @ Wo_pair  (K=128), DMA partial to DRAM
"""

import sys

for _p in ("/opt/trn_rl_repo", "/root/.axon_site/_ro/trn_rl_repo"):
    if _p not in sys.path:
        sys.path.insert(0, _p)

import numpy as np
import ml_dtypes

import concourse.bass as bass
import concourse.tile as tile
from concourse import bacc, mybir
from concourse.bass_utils import run_bass_kernel_spmd

BF16 = mybir.dt.bfloat16
F32 = mybir.dt.float32
AF = mybir.ActivationFunctionType

B, S, D, H, HD = 2, 2048, 1024, 16, 64
NCORES = 8
HPC = 4          # heads per core
NPAIR = 2        # head pairs per core
DC = D // 128    # 8 d-chunks
KC = S // 128    # 16 k-chunks
NQB = 4          # q blocks of 512
QB = 512
NQT = QB // 128  # q tiles per block

_CACHE = {}
_EXP_OPS = {}

# kc iterations (p=1 loops of blocks 1..3 only) whose h2=1 exp half runs on
# the vector engine via the custom poly-squaring chain. Those loops' vector
# queue is nearly empty and attn@v is globally deferred two iterations, so
# the 2-op chain's latency is absorbed. Must stay <= 13 (kc 14/15 h1 attn@v
# anchors the PSUM accumulation-group stop).
DVE_KCS = frozenset({6, 9, 13})


def _register_exp_ops():
    """Register two custom DVE ops computing exp(x*scale) as
    (1 + t + t^2/2 + t^3/6)^64 with t = x*scale/64 (rel err ~1.3e-5):
    ANT_EXPP_SEED evaluates the cubic, ANT_EXPP_SQ6 squares six times."""
    if _EXP_OPS:
        return
    import concourse.dve_ops as dops
    from concourse.dve_spec import Spec, Src0, C0, C1, C2, One, sq
    from concourse.dve_spec import lower as dve_lower
    from concourse.dve_uop import DveOpSpec

    t = Src0 * C0
    t2 = t * t
    t3 = t2 * t
    seed_spec = Spec(
        body=((One + t) + t2 * C1) + t3 * C2,
        reference=lambda in0, in1, s0, s1, imm2: (
            1.0 + in0 * s0 + (in0 * s0) ** 2 * s1 + (in0 * s0) ** 3 * imm2
        ).astype(np.float32),
    )
    b = Src0
    for _ in range(6):
        b = sq(b)
    sq6_spec = Spec(
        body=b,
        reference=lambda in0, in1, s0, s1, imm2: (
            in0.astype(np.float64) ** 64
        ).astype(np.float32),
    )

    for name, spec in (("ANT_EXPP_SEED", seed_spec), ("ANT_EXPP_SQ6", sq6_spec)):
        if name not in dops._SUB_OPCODE_FOR_NAME:
            row = max(dops._SUB_OPCODE_FOR_NAME.values()) + 1
            assert row < 0x20, "custom-DVE row field overflow"
            dops._SUB_OPCODE_FOR_NAME[name] = row
        shas = {}
        for ver in ("v3", "v4"):
            uops = dve_lower(spec, ver=ver)
            shas[ver] = DveOpSpec(
                name=name, opcode=dops._SUB_OPCODE_FOR_NAME[name], uops=uops,
                rd1_en=False,
            ).sha(ver)
        op = dops.DveOp(name, spec, subdim=False, uops_sha=shas)
        if all(o.name != name for o in dops.OPS):
            dops.OPS.append(op)
        dops.CUSTOM_DVE_SPECS[name] = spec
        _EXP_OPS[name] = op


def _build_program():
    _register_exp_ops()
    seed_op = _EXP_OPS["ANT_EXPP_SEED"]
    sq6_op = _EXP_OPS["ANT_EXPP_SQ6"]
    nc = bacc.Bacc("TRN2", target_bir_lowering=False, debug=False, num_devices=NCORES)

    xt = nc.dram_tensor("xt", [D, S], BF16, kind="ExternalInput").ap()
    et = nc.dram_tensor("et", [D, S], BF16, kind="ExternalInput").ap()
    wq = nc.dram_tensor("wq", [128, NPAIR, DC, 128], BF16, kind="ExternalInput").ap()
    wk = nc.dram_tensor("wk", [128, NPAIR, DC, 128], BF16, kind="ExternalInput").ap()
    wv = nc.dram_tensor("wv", [128, DC, 256], BF16, kind="ExternalInput").ap()
    wo = nc.dram_tensor("wo", [128, NPAIR, D], BF16, kind="ExternalInput").ap()
    bq = nc.dram_tensor("bq", [128, NPAIR], F32, kind="ExternalInput").ap()
    bk = nc.dram_tensor("bk", [128, NPAIR], F32, kind="ExternalInput").ap()
    # bf16 partials: host sums 4 partials per batch in f32; the bf16
    # rounding adds ~1.6e-3 rel err (vs 2e-2 budget) and halves output DMA
    out = nc.dram_tensor("out", [S, D], BF16, kind="ExternalOutput").ap()

    with tile.TileContext(nc) as tc:
        from contextlib import ExitStack

        with ExitStack() as ctx:
            wts = ctx.enter_context(tc.tile_pool(name="wts", bufs=1))
            big = ctx.enter_context(tc.tile_pool(name="big", bufs=1))

            # weights via gpsimd SWDGE; bulk activations via sync HWDGE in
            # strict need-order (splitting across queues starves the early
            # consumers — HBM bandwidth is the limit, not queue issue rate)
            wq_sb = wts.tile([128, NPAIR, DC, 128], BF16, name="wq_sb")
            wk_sb = wts.tile([128, NPAIR, DC, 128], BF16, name="wk_sb")
            wv_sb = wts.tile([128, DC, 256], BF16, name="wv_sb")
            wo_sb = wts.tile([128, NPAIR, D], BF16, name="wo_sb")
            bq_sb = wts.tile([128, NPAIR], F32, name="bq_sb")
            bk_sb = wts.tile([128, NPAIR], F32, name="bk_sb")
            for sb, dr in ((wk_sb, wk), (bk_sb, bk), (wq_sb, wq), (bq_sb, bq),
                           (wv_sb, wv), (wo_sb, wo)):
                nc.gpsimd.dma_start(sb, dr)

            xT = [big.tile([128, S], BF16, name=f"xT{d}") for d in range(DC)]
            eT = [big.tile([128, S], BF16, name=f"eT{d}") for d in range(DC)]
            # first eT block rides two queues (the 4 scalar triggers retire
            # well before the first ACTIVATE, so they cost no exp time);
            # everything else stays on sync in strict need-order
            sl0 = slice(0, QB)
            for d in range(DC):
                eng = nc.sync if d < 4 else nc.scalar
                eng.dma_start(eT[d][:, sl0], et[d * 128:(d + 1) * 128, sl0])
            for d in range(DC):
                nc.sync.dma_start(xT[d][:, sl0], xt[d * 128:(d + 1) * 128, sl0])
            for sb4 in range(1, NQB):
                sl = slice(sb4 * QB, (sb4 + 1) * QB)
                for d in range(DC):
                    nc.sync.dma_start(eT[d][:, sl], et[d * 128:(d + 1) * 128, sl])
            for sb4 in range(1, NQB):
                sl = slice(sb4 * QB, (sb4 + 1) * QB)
                for d in range(DC):
                    nc.sync.dma_start(xT[d][:, sl], xt[d * 128:(d + 1) * 128, sl])

            # ---- unified PSUM pools (8 banks total, live for whole kernel) ----
            dpool = ctx.enter_context(tc.tile_pool(name="dpool", bufs=4,
                                                   space="DRAM"))
            psc = ctx.enter_context(tc.tile_pool(name="psc", bufs=2, space="PSUM"))
            pou = ctx.enter_context(tc.tile_pool(name="pou", bufs=2, space="PSUM"))
            py = ctx.enter_context(tc.tile_pool(name="py", bufs=2, space="PSUM"))
            wk2 = ctx.enter_context(tc.tile_pool(name="wk2", bufs=2))
            expp = ctx.enter_context(tc.tile_pool(name="expp", bufs=6))
            exps = ctx.enter_context(tc.tile_pool(name="exps", bufs=2))

            # ---- projections; only kT[0] + v gate the first attention ----
            qT = [big.tile([128, S], BF16, name=f"qT{p}") for p in range(NPAIR)]
            kT = [big.tile([128, S], BF16, name=f"kT{p}") for p in range(NPAIR)]
            v = [big.tile([128, HPC, 65], BF16, name=f"v{s}") for s in range(KC)]

            def project_k_chunk(p, sb4):
                # deferred projections use the y-slot (idle during attention)
                sl = slice(sb4 * QB, (sb4 + 1) * QB)
                pk = py.tile([128, QB], F32, name="pk", tag="y")
                for d in range(DC):
                    nc.tensor.matmul(pk, wk_sb[:, p, d, :], eT[d][:, sl],
                                     start=(d == 0), stop=(d == DC - 1))
                nc.vector.tensor_scalar_add(kT[p][:, sl], pk, bk_sb[:, p:p + 1])

            def project_q(p, qb, tag="sc"):
                qsl = slice(qb * QB, (qb + 1) * QB)
                pool = psc if tag == "sc" else py
                pq = pool.tile([128, QB], F32, name="pq", tag=tag)
                for d in range(DC):
                    nc.tensor.matmul(pq, wq_sb[:, p, d, :], xT[d][:, qsl],
                                     start=(d == 0), stop=(d == DC - 1))
                nc.vector.tensor_scalar_add(qT[p][:, qsl], pq, bq_sb[:, p:p + 1])

            def project_v_tile(s):
                pv = py.tile([128, 256], F32, name="pv", tag="y")
                for d in range(DC):
                    nc.tensor.matmul(pv, eT[d][:, s * 128:(s + 1) * 128],
                                     wv_sb[:, d, :],
                                     start=(d == 0), stop=(d == DC - 1))
                nc.vector.tensor_copy(
                    v[s][:, :, 0:64], pv.rearrange("p (h e) -> p h e", h=HPC))
                nc.vector.memset(v[s][:, :, 64:65], 1.0)

            # PE warm-up: ~40 tiny matmuls on a zero tile run during the
            # input-DMA wait, so the HAM clock gate reaches 8/8 (2.4 GHz)
            # before the real projections start. memset on the (idle) vector
            # queue so the dummies are runnable immediately.
            # bridge must outlast the worst-case ~16-18us HBM delivery of the
            # first projections' inputs, else the PE idles >3.4us and the HAM
            # clock gate re-throttles the whole first projection wave to
            # 1.2 GHz; the N=256 tail extends coverage at low FIFO cost
            warm = wts.tile([128, 256], BF16, name="warm")
            nc.vector.memset(warm, 0.0)
            wdum = py.tile([64, 256], F32, name="wdum", tag="y")
            for _ in range(100):
                nc.tensor.matmul(wdum[:, 0:64], warm[:, 0:64], warm[:, 0:64])
            for _ in range(24):
                nc.tensor.matmul(wdum, warm[:, 0:64], warm)

            # minimal prologue: k00+q00 gate the first scores/exp; the v
            # tiles only gate the (now two-iterations-deferred) attn@v, so
            # they follow the q projection instead of preceding it
            project_k_chunk(0, 0)
            project_q(0, 0)
            for s in range(4):
                project_v_tile(s)

            def interject_qb0_p0(kc):
                if kc == 0:
                    project_k_chunk(0, 1)
                elif kc == 1:
                    project_v_tile(4); project_v_tile(5)
                elif kc == 2:
                    project_v_tile(6); project_v_tile(7)
                elif kc == 4:
                    project_k_chunk(0, 2)
                elif kc == 5:
                    project_v_tile(8); project_v_tile(9)
                elif kc == 6:
                    project_v_tile(10); project_v_tile(11)
                elif kc == 8:
                    project_k_chunk(0, 3)
                elif kc == 9:
                    project_v_tile(12); project_v_tile(13)
                elif kc == 10:
                    project_v_tile(14); project_v_tile(15)
                elif kc == 12:
                    project_k_chunk(1, 0)
                elif kc == 13:
                    project_q(1, 0)
                elif kc == 15:
                    project_q(0, 1, tag="y")

            def interject_qb0_p1(kc):
                if kc == 0:
                    project_k_chunk(1, 1)
                elif kc == 4:
                    project_k_chunk(1, 2)
                elif kc == 8:
                    project_k_chunk(1, 3)
                elif kc == 11:
                    project_q(1, 1, tag="y")

            # previous block's out-projection, spread one matmul per kc over
            # the following block's loops: group G=(qt,dc2) runs its two
            # pair-matmuls at p0-kcs 4+2G/5+2G (shifted +4 so the stk[1]
            # producer — the previous pair's normalize-mul, which is emitted
            # at the flush point below — has landed), copies trail one kc
            # behind; the last three groups spill into the p=1 loop.
            def outproj_mm(st, i, qbx, stk_pair):
                G, pp = divmod(i, 2)
                qt, dc2 = divmod(G, 2)
                dsl = slice(dc2 * 512, (dc2 + 1) * 512)
                tsl = slice(qt * 128, (qt + 1) * 128)
                if pp == 0:
                    st[G] = py.tile([128, 512], F32, name="yp", tag="y")
                nc.tensor.matmul(st[G], stk_pair[pp][:, tsl],
                                 wo_sb[:, pp, dsl],
                                 start=(pp == 0), stop=(pp == 1))

            def outproj_copy(st, G, qbx):
                qt, dc2 = divmod(G, 2)
                dsl = slice(dc2 * 512, (dc2 + 1) * 512)
                if dc2 == 0:
                    st[("ysb", qt)] = wk2.tile([128, D], BF16, name="ysb",
                                               tag="ysb", bufs=2)
                nc.vector.tensor_copy(st[("ysb", qt)][:, dsl], st[G])
                if dc2 == 1:
                    eng = nc.sync if qt % 2 == 0 else nc.gpsimd
                    eng.dma_start(out[qbx * QB + qt * 128:
                                      qbx * QB + (qt + 1) * 128, :],
                                  st[("ysb", qt)])

            # ---- attention + output projection ----
            # attn@v matmuls are deferred TWO kc iterations globally, spilling
            # across pair boundaries: otherwise the next pair's (independent)
            # first scores queue in the PE FIFO behind attnv(14)/(15), which
            # wait on the last exps — a ~2us stall at every pair boundary.
            # Each pair's normalization is emitted at its attnv(15) flush
            # point (inside the next pair's loop), overlapping the reciprocal/
            # broadcast chain with attention instead of the boundary.
            pend = []    # (global_kc, kc, ex, ou, p)
            norm_q = []  # pending per-pair normalization closures

            def make_norm(stk_p, ou, last_pair):
                def go():
                    for h2 in range(2):
                        # normalize: reciprocal of the rowsum row ->
                        # partition-broadcast via a DRAM bounce (step-0
                        # partition APs are DRAM-only) -> one multiply into
                        # the pair-stacked e-major tile. The LAST pair skips
                        # the osb bounce (reads ou PSUM directly): its slot
                        # release doesn't matter and the tail chain shortens.
                        if last_pair:
                            src = ou[h2]
                        else:
                            osb = wk2.tile([65, QB], F32, name=f"osb{h2}",
                                           tag=f"osb{h2}", bufs=2)
                            nc.vector.tensor_copy(osb, ou[h2])
                            src = osb
                        rr = wk2.tile([65, QB], F32, name="rr", tag="rr",
                                      bufs=4)
                        nc.vector.reciprocal_approx_fast(rr, src)
                        rrd = dpool.tile([1, QB], F32, name="rrd", tag="rrd")
                        nc.gpsimd.dma_start(rrd, rr[64:65, :])
                        rb = wk2.tile([64, QB], F32, name="rb", tag="rb",
                                      bufs=4)
                        rr_bcast = bass.AP(tensor=rrd.tensor, offset=rrd.offset,
                                           ap=[[0, 64]] + list(rrd.ap[1:]))
                        nc.gpsimd.dma_start(rb, rr_bcast)
                        nc.vector.tensor_mul(stk_p[h2 * 64:(h2 + 1) * 64, :],
                                             src[0:64, :], rb)
                return go

            def flush_pend(gkc_now):
                while pend and pend[0][0] <= gkc_now - 2:
                    _, pkc, pex, pou_, pp_ = pend.pop(0)
                    for h2 in range(2):
                        nc.tensor.matmul(pou_[h2], v[pkc][:, 2 * pp_ + h2, :],
                                         pex[:, h2, :],
                                         start=(pkc == 0), stop=(pkc == KC - 1))
                    if pkc == KC - 1 and norm_q:
                        norm_q.pop(0)()

            gkc_base = 0
            for qb in range(NQB):
                qsl = slice(qb * QB, (qb + 1) * QB)
                stk = [wk2.tile([128, QB], BF16, name=f"stk{p}", tag=f"stk{p}",
                                bufs=2) for p in range(NPAIR)]
                opst = {}
                for p in range(NPAIR):
                    if qb == 0:
                        interject = interject_qb0_p0 if p == 0 else interject_qb0_p1
                    elif p == 0:
                        def interject(kc, qb=qb, opst=opst):
                            if kc >= 4:
                                outproj_mm(opst, kc - 4, qb - 1, prev_stk)
                            if kc >= 6 and kc % 2 == 0:
                                outproj_copy(opst, (kc - 6) // 2, qb - 1)
                    else:
                        def interject(kc, p=p, qb=qb, opst=opst):
                            # spill of the previous block's out-projection
                            if kc <= 3:
                                outproj_mm(opst, 12 + kc, qb - 1, prev_stk)
                            if kc in (0, 2, 4):
                                outproj_copy(opst, 5 + kc // 2, qb - 1)
                            if qb < NQB - 1:
                                # next block's qT projections
                                if kc == 10:
                                    project_q(0, qb + 1, tag="y")
                                elif kc == 12:
                                    project_q(1, qb + 1, tag="y")
                    ou = [pou.tile([65, QB], F32, name=f"ou{h2}", tag="ou")
                          for h2 in range(2)]
                    for kc in range(KC):
                        ksl = slice(kc * 128, (kc + 1) * 128)
                        sc = psc.tile([128, 2, QB], F32, name="sc", tag="sc")
                        ex = expp.tile([128, 2, QB], BF16, name="ex", tag="ex")
                        for h2 in range(2):
                            hp = slice(h2 * 64, (h2 + 1) * 64)
                            nc.tensor.matmul(sc[:, h2, :], kT[p][hp, ksl],
                                             qT[p][hp, qsl])
                        # NOTE: offloading the h2=1 exp half to a custom
                        # vector-engine poly chain was tried extensively (even
                        # with the global attn@v deferral absorbing its
                        # latency) and measured net-negative every time; exp
                        # stays entirely on the scalar engine.
                        nc.scalar.activation(ex, sc, AF.Exp, scale=0.125)
                        flush_pend(gkc_base + kc)
                        pend.append((gkc_base + kc, kc, ex, ou, p))
                        if interject is not None:
                            interject(kc)
                        if qb == NQB - 1 and p == NPAIR - 1 and kc >= 6:
                            # the final pair loop has no interject work after
                            # the out-proj spill, dropping PE duty to ~56% —
                            # low enough for the HAM clock gate to re-throttle
                            # mid-loop and run the tail attn@v cold. Anchored
                            # filler matmuls (reading this kc's ex) keep the
                            # duty high. The py slot is free again by kc 6.
                            if kc == 6:
                                wdum3 = py.tile([64, 64], F32, name="wdum3",
                                                tag="y")
                            for _ in range(4):
                                nc.tensor.matmul(wdum3, ex[:, 0, 0:64],
                                                 ex[:, 0, 0:64])
                    gkc_base += KC
                    norm_q.append(make_norm(
                        stk[p], ou,
                        last_pair=(qb == NQB - 1 and p == NPAIR - 1)))
                prev_stk = stk

            # drain: the last two attn@v flushes + the final normalization,
            # with HAM-warming dummies (reading the last ex so the scheduler
            # cannot hoist them) covering the normalization chain
            tail_ex = pend[-1][2]
            flush_pend(gkc_base + KC)
            assert not pend and not norm_q
            wdum2 = py.tile([64, 256], F32, name="wdum2", tag="y")
            for _ in range(48):
                nc.tensor.matmul(wdum2, tail_ex[:, 0, 0:64],
                                 tail_ex[:, 0, 0:256])
            # final block's projection runs here in the tail
            qb = NQB - 1
            for qt in range(NQT):
                    tsl = slice(qt * 128, (qt + 1) * 128)
                    ysb = wk2.tile([128, D], BF16, name="ysb", tag="ysb", bufs=2)
                    for dc2 in range(2):
                        dsl = slice(dc2 * 512, (dc2 + 1) * 512)
                        yp = py.tile([128, 512], F32, name="yp", tag="y")
                        for p in range(NPAIR):
                            nc.tensor.matmul(yp, stk[p][:, tsl], wo_sb[:, p, dsl],
                                             start=(p == 0), stop=(p == NPAIR - 1))
                        if qb == NQB - 1:
                            # no exps follow the last block: split the copies
                            # across scalar+vector and DMA each half as soon
                            # as it lands, shortening the tail
                            ce = nc.scalar if dc2 == 0 else nc.vector
                            if ce is nc.scalar:
                                ce.copy(ysb[:, dsl], yp)
                            else:
                                ce.tensor_copy(ysb[:, dsl], yp)
                            nc.sync.dma_start(
                                out[qb * QB + qt * 128:
                                    qb * QB + (qt + 1) * 128, dsl],
                                ysb[:, dsl])
                        else:
                            nc.vector.tensor_copy(ysb[:, dsl], yp)
                    if qb < NQB - 1:
                        eng = nc.sync if qt % 2 == 0 else nc.gpsimd
                        eng.dma_start(out[qb * QB + qt * 128:
                                          qb * QB + (qt + 1) * 128, :], ysb)

    nc.compile()
    return nc


def _bf16(a):
    return np.ascontiguousarray(a.astype(ml_dtypes.bfloat16))


def _host_prep(inputs):
    x = np.asarray(inputs["x"], np.float32)
    enc = np.asarray(inputs["encoder_output"], np.float32)
    Wq = np.asarray(inputs["Wq"], np.float32)
    bq = np.asarray(inputs["bq"], np.float32)
    Wk = np.asarray(inputs["Wk"], np.float32)
    bk = np.asarray(inputs["bk"], np.float32)
    Wv = np.asarray(inputs["Wv"], np.float32)
    Wo = np.asarray(inputs["Wo"], np.float32)

    xt_b = [_bf16(x[b].T) for b in range(B)]
    et_b = [_bf16(enc[b].T) for b in range(B)]

    in_maps = []
    for c in range(NCORES):
        b = c // 4
        hb = HPC * (c % 4)

        wq_c = Wq[hb:hb + 4].reshape(2, 2, DC, 128, HD)  # [pair, hw, dc, dp, e]
        wq_c = wq_c.transpose(3, 0, 2, 1, 4).reshape(128, NPAIR, DC, 128)
        wk_c = Wk[hb:hb + 4].reshape(2, 2, DC, 128, HD)
        wk_c = wk_c.transpose(3, 0, 2, 1, 4).reshape(128, NPAIR, DC, 128)
        wv_c = Wv[hb:hb + 4].reshape(4, DC, 128, HD)
        wv_c = wv_c.transpose(2, 1, 0, 3).reshape(128, DC, 256)
        wo_c = Wo[hb * HD:(hb + 4) * HD].reshape(2, 2, HD, D)  # [pair, hw, e, d]
        wo_c = wo_c.transpose(1, 2, 0, 3).reshape(128, NPAIR, D)
        bq_c = bq[hb:hb + 4].reshape(2, 2, HD).transpose(1, 2, 0).reshape(128, NPAIR)
        bk_c = bk[hb:hb + 4].reshape(2, 2, HD).transpose(1, 2, 0).reshape(128, NPAIR)

        in_maps.append({
            "xt": xt_b[b],
            "et": et_b[b],
            "wq": _bf16(wq_c),
            "wk": _bf16(wk_c),
            "wv": _bf16(wv_c),
            "wo": _bf16(wo_c),
            "bq": np.ascontiguousarray(bq_c),
            "bk": np.ascontiguousarray(bk_c),
        })
    return in_maps


def kernel(**inputs):
    if "nc" not in _CACHE:
        _CACHE["nc"] = _build_program()
    nc = _CACHE["nc"]

    in_maps = _host_prep(inputs)
    res = None
    for attempt in range(3):
        try:
            res = run_bass_kernel_spmd(nc, in_maps, core_ids=list(range(NCORES)))
            break
        except Exception:
            if attempt == 2:
                raise
            import time
            time.sleep(5)
    _CACHE["last_results"] = res

    bv = np.asarray(inputs["bv"], np.float32)
    Wo = np.asarray(inputs["Wo"], np.float32)
    bo = np.asarray(inputs["bo"], np.float32)
    const_d = bo + np.einsum("he,hed->d", bv,
                             Wo.reshape(H, HD, D)).astype(np.float32)

    out = np.empty((B, S, D), np.float32)
    for b in range(B):
        acc = res.results[4 * b]["out"].astype(np.float32).copy()
        for c in range(4 * b + 1, 4 * b + 4):
            acc += res.results[c]["out"]
        out[b] = acc + const_d
    return out



# revision 69
# speedup vs baseline: 1.0098x; 1.0098x over previous
"""Trainium2 Bass kernel for CrossMultiHeadedSelfAttention.

Problem: B=2, SQ=SK=2048, D=1024, H=16, HD=64 cross-attention
  q = x @ Wq + bq ; k = enc @ Wk + bk ; v = enc @ Wv + bv   (per head)
  out = softmax(q k^T / sqrt(HD)) v  -> concat heads -> @ Wo + bo

Sharding: 8 cores = 2 batches x 4 head-groups (4 heads per core).
Each core computes a partial output projection over its 4 heads; the host
sums the 4 partials per batch and adds the constant term
(bo + sum_h bv_h @ Wo_h, exact because softmax rows sum to 1).

Device-side math (per core, bf16 matmuls, f32 accumulation):
  - x/enc are pre-transposed AND pre-cast to bf16 on the host, so xT/encT
    d-major tiles load with fully contiguous DMA
  - qT/kT in [head-pair e (128) x seq] layout, bias via per-partition
    tensor_scalar add; v in natural [s, 4*65] layout with a ones column
    per head (gives softmax row-sums for free in the attn@v matmul)
  - scoresT chunk = kT_h.T @ qT_h  -> exp (scale=1/8, no max subtraction:
    scores ~ N(0,1), |s|max ~ 6 so exp is safe in f32/bf16)
  - outU = v'_h.T @ expT  ([65 x 512] in PSUM, row 64 = softmax row-sum)
  - normalize without any transpose: reciprocal of row 64 -> tiny
    partition-broadcast DMA [1,512]->[64,512] -> one tensor_mul writes the
    normalized e-major tile into the pair-stacked stk buffer
  - y = sum_pairs stk_pair.T @ Wo_pair  (K=128), DMA partial to DRAM
"""

import sys

for _p in ("/opt/trn_rl_repo", "/root/.axon_site/_ro/trn_rl_repo"):
    if _p not in sys.path:
        sys.path.insert(0, _p)

import numpy as np
import ml_dtypes

import concourse.bass as bass
import concourse.tile as tile
from concourse import bacc, mybir
from concourse.bass_utils import run_bass_kernel_spmd

BF16 = mybir.dt.bfloat16
F32 = mybir.dt.float32
AF = mybir.ActivationFunctionType

B, S, D, H, HD = 2, 2048, 1024, 16, 64
NCORES = 8
HPC = 4          # heads per core
NPAIR = 2        # head pairs per core
DC = D // 128    # 8 d-chunks
KC = S // 128    # 16 k-chunks
NQB = 4          # q blocks of 512
QB = 512
NQT = QB // 128  # q tiles per block

_CACHE = {}
_EXP_OPS = {}

# kc iterations (p=1 loops of blocks 1..3 only) whose h2=1 exp half runs on
# the vector engine via the custom poly-squaring chain. Those loops' vector
# queue is nearly empty and attn@v is globally deferred two iterations, so
# the 2-op chain's latency is absorbed. Must stay <= 13 (kc 14/15 h1 attn@v
# anchors the PSUM accumulation-group stop).
DVE_KCS = frozenset({6, 9, 13})


def _register_exp_ops():
    """Register two custom DVE ops computing exp(x*scale) as
    (1 + t + t^2/2 + t^3/6)^64 with t = x*scale/64 (rel err ~1.3e-5):
    ANT_EXPP_SEED evaluates the cubic, ANT_EXPP_SQ6 squares six times."""
    if _EXP_OPS:
        return
    import concourse.dve_ops as dops
    from concourse.dve_spec import Spec, Src0, C0, C1, C2, One, sq
    from concourse.dve_spec import lower as dve_lower
    from concourse.dve_uop import DveOpSpec

    t = Src0 * C0
    t2 = t * t
    t3 = t2 * t
    seed_spec = Spec(
        body=((One + t) + t2 * C1) + t3 * C2,
        reference=lambda in0, in1, s0, s1, imm2: (
            1.0 + in0 * s0 + (in0 * s0) ** 2 * s1 + (in0 * s0) ** 3 * imm2
        ).astype(np.float32),
    )
    b = Src0
    for _ in range(6):
        b = sq(b)
    sq6_spec = Spec(
        body=b,
        reference=lambda in0, in1, s0, s1, imm2: (
            in0.astype(np.float64) ** 64
        ).astype(np.float32),
    )

    for name, spec in (("ANT_EXPP_SEED", seed_spec), ("ANT_EXPP_SQ6", sq6_spec)):
        if name not in dops._SUB_OPCODE_FOR_NAME:
            row = max(dops._SUB_OPCODE_FOR_NAME.values()) + 1
            assert row < 0x20, "custom-DVE row field overflow"
            dops._SUB_OPCODE_FOR_NAME[name] = row
        shas = {}
        for ver in ("v3", "v4"):
            uops = dve_lower(spec, ver=ver)
            shas[ver] = DveOpSpec(
                name=name, opcode=dops._SUB_OPCODE_FOR_NAME[name], uops=uops,
                rd1_en=False,
            ).sha(ver)
        op = dops.DveOp(name, spec, subdim=False, uops_sha=shas)
        if all(o.name != name for o in dops.OPS):
            dops.OPS.append(op)
        dops.CUSTOM_DVE_SPECS[name] = spec
        _EXP_OPS[name] = op


def _build_program():
    _register_exp_ops()
    seed_op = _EXP_OPS["ANT_EXPP_SEED"]
    sq6_op = _EXP_OPS["ANT_EXPP_SQ6"]
    nc = bacc.Bacc("TRN2", target_bir_lowering=False, debug=False, num_devices=NCORES)

    xt = nc.dram_tensor("xt", [D, S], BF16, kind="ExternalInput").ap()
    et = nc.dram_tensor("et", [D, S], BF16, kind="ExternalInput").ap()
    wq = nc.dram_tensor("wq", [128, NPAIR, DC, 128], BF16, kind="ExternalInput").ap()
    wk = nc.dram_tensor("wk", [128, NPAIR, DC, 128], BF16, kind="ExternalInput").ap()
    wv = nc.dram_tensor("wv", [128, DC, 256], BF16, kind="ExternalInput").ap()
    wo = nc.dram_tensor("wo", [128, NPAIR, D], BF16, kind="ExternalInput").ap()
    bq = nc.dram_tensor("bq", [128, NPAIR], F32, kind="ExternalInput").ap()
    bk = nc.dram_tensor("bk", [128, NPAIR], F32, kind="ExternalInput").ap()
    # bf16 partials: host sums 4 partials per batch in f32; the bf16
    # rounding adds ~1.6e-3 rel err (vs 2e-2 budget) and halves output DMA
    out = nc.dram_tensor("out", [S, D], BF16, kind="ExternalOutput").ap()

    with tile.TileContext(nc) as tc:
        from contextlib import ExitStack

        with ExitStack() as ctx:
            wts = ctx.enter_context(tc.tile_pool(name="wts", bufs=1))
            big = ctx.enter_context(tc.tile_pool(name="big", bufs=1))

            # weights via gpsimd SWDGE; bulk activations via sync HWDGE in
            # strict need-order (splitting across queues starves the early
            # consumers — HBM bandwidth is the limit, not queue issue rate)
            wq_sb = wts.tile([128, NPAIR, DC, 128], BF16, name="wq_sb")
            wk_sb = wts.tile([128, NPAIR, DC, 128], BF16, name="wk_sb")
            wv_sb = wts.tile([128, DC, 256], BF16, name="wv_sb")
            wo_sb = wts.tile([128, NPAIR, D], BF16, name="wo_sb")
            bq_sb = wts.tile([128, NPAIR], F32, name="bq_sb")
            bk_sb = wts.tile([128, NPAIR], F32, name="bk_sb")
            for sb, dr in ((wk_sb, wk), (bk_sb, bk), (wq_sb, wq), (bq_sb, bq),
                           (wv_sb, wv), (wo_sb, wo)):
                nc.gpsimd.dma_start(sb, dr)

            xT = [big.tile([128, S], BF16, name=f"xT{d}") for d in range(DC)]
            eT = [big.tile([128, S], BF16, name=f"eT{d}") for d in range(DC)]
            # first eT block rides two queues (the 4 scalar triggers retire
            # well before the first ACTIVATE, so they cost no exp time);
            # everything else stays on sync in strict need-order
            sl0 = slice(0, QB)
            for d in range(DC):
                eng = nc.sync if d < 4 else nc.scalar
                eng.dma_start(eT[d][:, sl0], et[d * 128:(d + 1) * 128, sl0])
            for d in range(DC):
                nc.sync.dma_start(xT[d][:, sl0], xt[d * 128:(d + 1) * 128, sl0])
            for sb4 in range(1, NQB):
                sl = slice(sb4 * QB, (sb4 + 1) * QB)
                for d in range(DC):
                    nc.sync.dma_start(eT[d][:, sl], et[d * 128:(d + 1) * 128, sl])
            for sb4 in range(1, NQB):
                sl = slice(sb4 * QB, (sb4 + 1) * QB)
                for d in range(DC):
                    nc.sync.dma_start(xT[d][:, sl], xt[d * 128:(d + 1) * 128, sl])

            # ---- unified PSUM pools (8 banks total, live for whole kernel) ----
            dpool = ctx.enter_context(tc.tile_pool(name="dpool", bufs=4,
                                                   space="DRAM"))
            psc = ctx.enter_context(tc.tile_pool(name="psc", bufs=2, space="PSUM"))
            pou = ctx.enter_context(tc.tile_pool(name="pou", bufs=2, space="PSUM"))
            py = ctx.enter_context(tc.tile_pool(name="py", bufs=2, space="PSUM"))
            wk2 = ctx.enter_context(tc.tile_pool(name="wk2", bufs=2))
            expp = ctx.enter_context(tc.tile_pool(name="expp", bufs=6))
            exps = ctx.enter_context(tc.tile_pool(name="exps", bufs=2))

            # ---- projections; only kT[0] + v gate the first attention ----
            qT = [big.tile([128, S], BF16, name=f"qT{p}") for p in range(NPAIR)]
            kT = [big.tile([128, S], BF16, name=f"kT{p}") for p in range(NPAIR)]
            v = [big.tile([128, HPC, 65], BF16, name=f"v{s}") for s in range(KC)]

            def project_k_chunk(p, sb4):
                # deferred projections use the y-slot (idle during attention)
                sl = slice(sb4 * QB, (sb4 + 1) * QB)
                pk = py.tile([128, QB], F32, name="pk", tag="y")
                for d in range(DC):
                    nc.tensor.matmul(pk, wk_sb[:, p, d, :], eT[d][:, sl],
                                     start=(d == 0), stop=(d == DC - 1))
                nc.vector.tensor_scalar_add(kT[p][:, sl], pk, bk_sb[:, p:p + 1])

            def project_q(p, qb, tag="sc"):
                qsl = slice(qb * QB, (qb + 1) * QB)
                pool = psc if tag == "sc" else py
                pq = pool.tile([128, QB], F32, name="pq", tag=tag)
                for d in range(DC):
                    nc.tensor.matmul(pq, wq_sb[:, p, d, :], xT[d][:, qsl],
                                     start=(d == 0), stop=(d == DC - 1))
                nc.vector.tensor_scalar_add(qT[p][:, qsl], pq, bq_sb[:, p:p + 1])

            def project_v_tile(s):
                pv = py.tile([128, 256], F32, name="pv", tag="y")
                for d in range(DC):
                    nc.tensor.matmul(pv, eT[d][:, s * 128:(s + 1) * 128],
                                     wv_sb[:, d, :],
                                     start=(d == 0), stop=(d == DC - 1))
                nc.vector.tensor_copy(
                    v[s][:, :, 0:64], pv.rearrange("p (h e) -> p h e", h=HPC))
                nc.vector.memset(v[s][:, :, 64:65], 1.0)

            # PE warm-up: ~40 tiny matmuls on a zero tile run during the
            # input-DMA wait, so the HAM clock gate reaches 8/8 (2.4 GHz)
            # before the real projections start. memset on the (idle) vector
            # queue so the dummies are runnable immediately.
            # bridge must outlast the worst-case ~16-18us HBM delivery of the
            # first projections' inputs, else the PE idles >3.4us and the HAM
            # clock gate re-throttles the whole first projection wave to
            # 1.2 GHz; the N=256 tail extends coverage at low FIFO cost
            warm = wts.tile([128, 256], BF16, name="warm")
            nc.vector.memset(warm, 0.0)
            wdum = py.tile([64, 256], F32, name="wdum", tag="y")
            for _ in range(100):
                nc.tensor.matmul(wdum[:, 0:64], warm[:, 0:64], warm[:, 0:64])
            for _ in range(40):
                nc.tensor.matmul(wdum, warm[:, 0:64], warm)

            # minimal prologue: k00+q00 gate the first scores/exp; the v
            # tiles only gate the (now two-iterations-deferred) attn@v, so
            # they follow the q projection instead of preceding it
            project_k_chunk(0, 0)
            project_q(0, 0)
            for s in range(4):
                project_v_tile(s)

            def interject_qb0_p0(kc):
                if kc == 0:
                    project_k_chunk(0, 1)
                elif kc == 1:
                    project_v_tile(4); project_v_tile(5)
                elif kc == 2:
                    project_v_tile(6); project_v_tile(7)
                elif kc == 4:
                    project_k_chunk(0, 2)
                elif kc == 5:
                    project_v_tile(8); project_v_tile(9)
                elif kc == 6:
                    project_v_tile(10); project_v_tile(11)
                elif kc == 8:
                    project_k_chunk(0, 3)
                elif kc == 9:
                    project_v_tile(12); project_v_tile(13)
                elif kc == 10:
                    project_v_tile(14); project_v_tile(15)
                elif kc == 12:
                    project_k_chunk(1, 0)
                elif kc == 13:
                    project_q(1, 0)
                elif kc == 15:
                    project_q(0, 1, tag="y")

            def interject_qb0_p1(kc):
                if kc == 0:
                    project_k_chunk(1, 1)
                elif kc == 4:
                    project_k_chunk(1, 2)
                elif kc == 8:
                    project_k_chunk(1, 3)
                elif kc == 11:
                    project_q(1, 1, tag="y")

            # previous block's out-projection, spread one matmul per kc over
            # the following block's loops: group G=(qt,dc2) runs its two
            # pair-matmuls at p0-kcs 4+2G/5+2G (shifted +4 so the stk[1]
            # producer — the previous pair's normalize-mul, which is emitted
            # at the flush point below — has landed), copies trail one kc
            # behind; the last three groups spill into the p=1 loop.
            def outproj_mm(st, i, qbx, stk_pair):
                G, pp = divmod(i, 2)
                qt, dc2 = divmod(G, 2)
                dsl = slice(dc2 * 512, (dc2 + 1) * 512)
                tsl = slice(qt * 128, (qt + 1) * 128)
                if pp == 0:
                    st[G] = py.tile([128, 512], F32, name="yp", tag="y")
                nc.tensor.matmul(st[G], stk_pair[pp][:, tsl],
                                 wo_sb[:, pp, dsl],
                                 start=(pp == 0), stop=(pp == 1))

            def outproj_copy(st, G, qbx):
                qt, dc2 = divmod(G, 2)
                dsl = slice(dc2 * 512, (dc2 + 1) * 512)
                if dc2 == 0:
                    st[("ysb", qt)] = wk2.tile([128, D], BF16, name="ysb",
                                               tag="ysb", bufs=2)
                nc.vector.tensor_copy(st[("ysb", qt)][:, dsl], st[G])
                if dc2 == 1:
                    eng = nc.sync if qt % 2 == 0 else nc.gpsimd
                    eng.dma_start(out[qbx * QB + qt * 128:
                                      qbx * QB + (qt + 1) * 128, :],
                                  st[("ysb", qt)])

            # ---- attention + output projection ----
            # attn@v matmuls are deferred TWO kc iterations globally, spilling
            # across pair boundaries: otherwise the next pair's (independent)
            # first scores queue in the PE FIFO behind attnv(14)/(15), which
            # wait on the last exps — a ~2us stall at every pair boundary.
            # Each pair's normalization is emitted at its attnv(15) flush
            # point (inside the next pair's loop), overlapping the reciprocal/
            # broadcast chain with attention instead of the boundary.
            pend = []    # (global_kc, kc, ex, ou, p)
            norm_q = []  # pending per-pair normalization closures

            def make_norm(stk_p, ou, last_pair):
                def go():
                    for h2 in range(2):
                        # normalize: reciprocal of the rowsum row ->
                        # partition-broadcast via a DRAM bounce (step-0
                        # partition APs are DRAM-only) -> one multiply into
                        # the pair-stacked e-major tile. The LAST pair skips
                        # the osb bounce (reads ou PSUM directly): its slot
                        # release doesn't matter and the tail chain shortens.
                        if last_pair:
                            src = ou[h2]
                        else:
                            osb = wk2.tile([65, QB], F32, name=f"osb{h2}",
                                           tag=f"osb{h2}", bufs=2)
                            nc.vector.tensor_copy(osb, ou[h2])
                            src = osb
                        rr = wk2.tile([65, QB], F32, name="rr", tag="rr",
                                      bufs=4)
                        nc.vector.reciprocal_approx_fast(rr, src)
                        rrd = dpool.tile([1, QB], F32, name="rrd", tag="rrd")
                        nc.gpsimd.dma_start(rrd, rr[64:65, :])
                        rb = wk2.tile([64, QB], F32, name="rb", tag="rb",
                                      bufs=4)
                        rr_bcast = bass.AP(tensor=rrd.tensor, offset=rrd.offset,
                                           ap=[[0, 64]] + list(rrd.ap[1:]))
                        nc.gpsimd.dma_start(rb, rr_bcast)
                        nc.vector.tensor_mul(stk_p[h2 * 64:(h2 + 1) * 64, :],
                                             src[0:64, :], rb)
                return go

            def flush_pend(gkc_now):
                while pend and pend[0][0] <= gkc_now - 2:
                    _, pkc, pex, pou_, pp_ = pend.pop(0)
                    for h2 in range(2):
                        nc.tensor.matmul(pou_[h2], v[pkc][:, 2 * pp_ + h2, :],
                                         pex[:, h2, :],
                                         start=(pkc == 0), stop=(pkc == KC - 1))
                    if pkc == KC - 1 and norm_q:
                        norm_q.pop(0)()

            gkc_base = 0
            for qb in range(NQB):
                qsl = slice(qb * QB, (qb + 1) * QB)
                stk = [wk2.tile([128, QB], BF16, name=f"stk{p}", tag=f"stk{p}",
                                bufs=2) for p in range(NPAIR)]
                opst = {}
                for p in range(NPAIR):
                    if qb == 0:
                        interject = interject_qb0_p0 if p == 0 else interject_qb0_p1
                    elif p == 0:
                        def interject(kc, qb=qb, opst=opst):
                            if kc >= 4:
                                outproj_mm(opst, kc - 4, qb - 1, prev_stk)
                            if kc >= 6 and kc % 2 == 0:
                                outproj_copy(opst, (kc - 6) // 2, qb - 1)
                    else:
                        def interject(kc, p=p, qb=qb, opst=opst):
                            # spill of the previous block's out-projection
                            if kc <= 3:
                                outproj_mm(opst, 12 + kc, qb - 1, prev_stk)
                            if kc in (0, 2, 4):
                                outproj_copy(opst, 5 + kc // 2, qb - 1)
                            if qb < NQB - 1:
                                # next block's qT projections
                                if kc == 10:
                                    project_q(0, qb + 1, tag="y")
                                elif kc == 12:
                                    project_q(1, qb + 1, tag="y")
                    ou = [pou.tile([65, QB], F32, name=f"ou{h2}", tag="ou")
                          for h2 in range(2)]
                    for kc in range(KC):
                        ksl = slice(kc * 128, (kc + 1) * 128)
                        sc = psc.tile([128, 2, QB], F32, name="sc", tag="sc")
                        ex = expp.tile([128, 2, QB], BF16, name="ex", tag="ex")
                        for h2 in range(2):
                            hp = slice(h2 * 64, (h2 + 1) * 64)
                            nc.tensor.matmul(sc[:, h2, :], kT[p][hp, ksl],
                                             qT[p][hp, qsl])
                        # NOTE: offloading the h2=1 exp half to a custom
                        # vector-engine poly chain was tried extensively (even
                        # with the global attn@v deferral absorbing its
                        # latency) and measured net-negative every time; exp
                        # stays entirely on the scalar engine.
                        nc.scalar.activation(ex, sc, AF.Exp, scale=0.125)
                        flush_pend(gkc_base + kc)
                        pend.append((gkc_base + kc, kc, ex, ou, p))
                        if interject is not None:
                            interject(kc)
                        if qb == NQB - 1 and p == NPAIR - 1 and kc >= 6:
                            # the final pair loop has no interject work after
                            # the out-proj spill, dropping PE duty to ~56% —
                            # low enough for the HAM clock gate to re-throttle
                            # mid-loop and run the tail attn@v cold. Anchored
                            # filler matmuls (reading this kc's ex) keep the
                            # duty high. The py slot is free again by kc 6.
                            if kc == 6:
                                wdum3 = py.tile([64, 64], F32, name="wdum3",
                                                tag="y")
                            for _ in range(4):
                                nc.tensor.matmul(wdum3, ex[:, 0, 0:64],
                                                 ex[:, 0, 0:64])
                    gkc_base += KC
                    norm_q.append(make_norm(
                        stk[p], ou,
                        last_pair=(qb == NQB - 1 and p == NPAIR - 1)))
                prev_stk = stk

            # drain: the last two attn@v flushes + the final normalization,
            # with HAM-warming dummies (reading the last ex so the scheduler
            # cannot hoist them) covering the normalization chain
            tail_ex = pend[-1][2]
            flush_pend(gkc_base + KC)
            assert not pend and not norm_q
            wdum2 = py.tile([64, 256], F32, name="wdum2", tag="y")
            for _ in range(48):
                nc.tensor.matmul(wdum2, tail_ex[:, 0, 0:64],
                                 tail_ex[:, 0, 0:256])
            # final block's projection runs here in the tail
            qb = NQB - 1
            for qt in range(NQT):
                    tsl = slice(qt * 128, (qt + 1) * 128)
                    ysb = wk2.tile([128, D], BF16, name="ysb", tag="ysb", bufs=2)
                    for dc2 in range(2):
                        dsl = slice(dc2 * 512, (dc2 + 1) * 512)
                        yp = py.tile([128, 512], F32, name="yp", tag="y")
                        for p in range(NPAIR):
                            nc.tensor.matmul(yp, stk[p][:, tsl], wo_sb[:, p, dsl],
                                             start=(p == 0), stop=(p == NPAIR - 1))
                        if qb == NQB - 1:
                            # no exps follow the last block: split the copies
                            # across scalar+vector and DMA each half as soon
                            # as it lands, shortening the tail
                            ce = nc.scalar if dc2 == 0 else nc.vector
                            if ce is nc.scalar:
                                ce.copy(ysb[:, dsl], yp)
                            else:
                                ce.tensor_copy(ysb[:, dsl], yp)
                            nc.sync.dma_start(
                                out[qb * QB + qt * 128:
                                    qb * QB + (qt + 1) * 128, dsl],
                                ysb[:, dsl])
                        else:
                            nc.vector.tensor_copy(ysb[:, dsl], yp)
                    if qb < NQB - 1:
                        eng = nc.sync if qt % 2 == 0 else nc.gpsimd
                        eng.dma_start(out[qb * QB + qt * 128:
                                          qb * QB + (qt + 1) * 128, :], ysb)

    nc.compile()
    return nc


def _bf16(a):
    return np.ascontiguousarray(a.astype(ml_dtypes.bfloat16))


def _host_prep(inputs):
    x = np.asarray(inputs["x"], np.float32)
    enc = np.asarray(inputs["encoder_output"], np.float32)
    Wq = np.asarray(inputs["Wq"], np.float32)
    bq = np.asarray(inputs["bq"], np.float32)
    Wk = np.asarray(inputs["Wk"], np.float32)
    bk = np.asarray(inputs["bk"], np.float32)
    Wv = np.asarray(inputs["Wv"], np.float32)
    Wo = np.asarray(inputs["Wo"], np.float32)

    xt_b = [_bf16(x[b].T) for b in range(B)]
    et_b = [_bf16(enc[b].T) for b in range(B)]

    in_maps = []
    for c in range(NCORES):
        b = c // 4
        hb = HPC * (c % 4)

        wq_c = Wq[hb:hb + 4].reshape(2, 2, DC, 128, HD)  # [pair, hw, dc, dp, e]
        wq_c = wq_c.transpose(3, 0, 2, 1, 4).reshape(128, NPAIR, DC, 128)
        wk_c = Wk[hb:hb + 4].reshape(2, 2, DC, 128, HD)
        wk_c = wk_c.transpose(3, 0, 2, 1, 4).reshape(128, NPAIR, DC, 128)
        wv_c = Wv[hb:hb + 4].reshape(4, DC, 128, HD)
        wv_c = wv_c.transpose(2, 1, 0, 3).reshape(128, DC, 256)
        wo_c = Wo[hb * HD:(hb + 4) * HD].reshape(2, 2, HD, D)  # [pair, hw, e, d]
        wo_c = wo_c.transpose(1, 2, 0, 3).reshape(128, NPAIR, D)
        bq_c = bq[hb:hb + 4].reshape(2, 2, HD).transpose(1, 2, 0).reshape(128, NPAIR)
        bk_c = bk[hb:hb + 4].reshape(2, 2, HD).transpose(1, 2, 0).reshape(128, NPAIR)

        in_maps.append({
            "xt": xt_b[b],
            "et": et_b[b],
            "wq": _bf16(wq_c),
            "wk": _bf16(wk_c),
            "wv": _bf16(wv_c),
            "wo": _bf16(wo_c),
            "bq": np.ascontiguousarray(bq_c),
            "bk": np.ascontiguousarray(bk_c),
        })
    return in_maps


def kernel(**inputs):
    if "nc" not in _CACHE:
        _CACHE["nc"] = _build_program()
    nc = _CACHE["nc"]

    in_maps = _host_prep(inputs)
    res = None
    for attempt in range(3):
        try:
            res = run_bass_kernel_spmd(nc, in_maps, core_ids=list(range(NCORES)))
            break
        except Exception:
            if attempt == 2:
                raise
            import time
            time.sleep(5)
    _CACHE["last_results"] = res

    bv = np.asarray(inputs["bv"], np.float32)
    Wo = np.asarray(inputs["Wo"], np.float32)
    bo = np.asarray(inputs["bo"], np.float32)
    const_d = bo + np.einsum("he,hed->d", bv,
                             Wo.reshape(H, HD, D)).astype(np.float32)

    out = np.empty((B, S, D), np.float32)
    for b in range(B):
        acc = res.results[4 * b]["out"].astype(np.float32).copy()
        for c in range(4 * b + 1, 4 * b + 4):
            acc += res.results[c]["out"]
        out[b] = acc + const_d
    return out

